# revision 10
# baseline (speedup 1.0000x reference)
"""4D Conv-MLP (conv3^4 -> ReLU -> conv3^4) on 8 Trainium2 NeuronCores.

Sharding: core = b*4 + j  (batch b in {0,1}, H-slab j in {0..3}, 8 output rows
each). Each core computes its output slab independently: conv1 is recomputed on
a 1-row h halo (10 h rows from 12 x rows), so no cross-core communication is
needed. One SPMD program for all cores; per-core boundary behavior is driven by
data (host-zeroed x halos + h halo-row masks).

On-chip algorithm (implicit GEMM over the 81 taps, fp16 operands, fp32 PSUM):
  - x is stored channel-on-partition as a zero-padded flat plane per t:
    [18 D][12 H][34 W] (+1 lead pad), duplicated on partitions 64..127 shifted
    by one element so each K=128 matmul contracts two W-taps at once.
  - conv1: per (t, d): 340-column matmuls accumulating (kt,ku,kv) pair+single
    taps; ReLU+bias on the Scalar engine writes fp16 h (pads skipped).
  - conv2: per (t, d-pair): 512-column matmuls over all valid taps with
    K=128; bias added on DVE; fp32 result DMAd out.
"""

import numpy as np

B, C_IN, C_HID, C_OUT = 2, 64, 128, 64
T, D, H, W = 4, 16, 32, 32
NCORES, NJ = 8, 4
SH = H // NJ          # 8 out rows per slab
XH = SH + 4           # 12 x rows per slab
HHH = SH + 2          # 10 h rows per slab
XROW = 34             # padded W
XDP = 12 * XROW       # 408
XP = 1 + 16 * XDP + 7   # x plane size (real D rows only) = 6536
HD, HW_ = 18, 34
HP = HD * HHH * HW_   # h plane = 6120
N1 = HHH * XROW       # conv1 run = 340
N2 = 512              # conv2 run (2 d-rows)

_cache = {}
import os
PAIR_CONV2 = os.environ.get('K_PAIR_CONV2', '1') == '1'
PAIR_SINGLES = os.environ.get('K_PAIR_SINGLES', '1') == '1'


def _t_taps(t):
    return [kt for kt in range(3) if 0 <= t + kt - 1 < T]


def _g27(kt, ku, kv):
    return (kt * 3 + ku) * 3 + kv


def _g81(kt, ku, kv, kw):
    return ((kt * 3 + ku) * 3 + kv) * 3 + kw


def _make_host_arrays(x, w1, b1, w2, b2):
    x = np.asarray(x, np.float32)
    Xs, MTs, MBs = [], [], []
    for core in range(NCORES):
        b, j = divmod(core, NJ)
        h0 = SH * j
        slab = np.zeros((C_IN, T, D, XH, W), np.float32)
        lo, hi = h0 - 2, h0 + 10
        slo, shi = max(lo, 0), min(hi, H)
        slab[:, :, :, slo - lo:shi - lo, :] = x[b, :, :, :, slo:shi, :]
        plane = np.zeros((C_IN, T, D, XH, XROW), np.float32)
        plane[:, :, :, :, 1:33] = slab
        flat = plane.reshape(C_IN, T, D * XDP)
        X = np.zeros((C_IN, T, XP), np.float16)
        X[:, :, 1:1 + D * XDP] = flat
        Xs.append(X)
        MTs.append(np.full((128, 1), 0.0 if j == 0 else 1.0, np.float32))
        MBs.append(np.full((128, 1), 0.0 if j == NJ - 1 else 1.0, np.float32))

    w1 = np.asarray(w1, np.float32)
    w2 = np.asarray(w2, np.float32)
    W1P = np.zeros((128, 27, 128), np.float16)   # tileA pairs: (kv,kw=0)+(kv,kw=1)
    W1PB = np.zeros((128, 9, 128), np.float16)   # tileB pair: (0,2)+(1,2)
    W1S = np.zeros((128, 9, 128), np.float16)    # tileB-top single: (2,2)
    for kt in range(3):
        for ku in range(3):
            g9 = kt * 3 + ku
            W1PB[:64, g9, :] = w1[:, :, kt, ku, 0, 2].T
            W1PB[64:, g9, :] = w1[:, :, kt, ku, 1, 2].T
            W1S[:64, g9, :] = w1[:, :, kt, ku, 2, 2].T
            for kv in range(3):
                g = _g27(kt, ku, kv)
                W1P[:64, g, :] = w1[:, :, kt, ku, kv, 0].T
                W1P[64:, g, :] = w1[:, :, kt, ku, kv, 1].T
    W2 = np.zeros((128, 81, 64), np.float16)
    for kt in range(3):
        for ku in range(3):
            for kv in range(3):
                for kw in range(3):
                    gi = _g81(kt, ku, kv, kw)
                    W2[:, gi, :] = w2[:, :, kt, ku, kv, kw].T
    return dict(X=Xs, MT=MTs, MB=MBs,
                W1P=W1P.reshape(128, 27 * 128), W1PB=W1PB.reshape(128, 9 * 128),
                W1S=W1S.reshape(128, 9 * 128),
                W2=W2.reshape(128, 81 * 64),
                B1=np.asarray(b1, np.float32).reshape(128, 1),
                B2=np.asarray(b2, np.float32).reshape(64, 1))


def _build_module():
    import concourse.bass as bass
    import concourse.tile as tile
    from concourse import bacc, mybir

    fp16 = mybir.dt.float16
    fp32 = mybir.dt.float32

    nc = bacc.Bacc("TRN2", target_bir_lowering=False, debug=False, num_devices=1)
    x_d = nc.dram_tensor("x", [64, T, XP], fp16, kind="ExternalInput")
    w1p_d = nc.dram_tensor("w1p", [128, 27 * 128], fp16, kind="ExternalInput")
    w1pb_d = nc.dram_tensor("w1pb", [128, 9 * 128], fp16, kind="ExternalInput")
    w1s_d = nc.dram_tensor("w1s", [128, 9 * 128], fp16, kind="ExternalInput")
    w2_d = nc.dram_tensor("w2", [128, 81 * 64], fp16, kind="ExternalInput")
    b1_d = nc.dram_tensor("b1", [128, 1], fp32, kind="ExternalInput")
    b2_d = nc.dram_tensor("b2", [64, 1], fp32, kind="ExternalInput")
    mt_d = nc.dram_tensor("mt", [128, 1], fp32, kind="ExternalInput")
    mb_d = nc.dram_tensor("mb", [128, 1], fp32, kind="ExternalInput")
    y_d = nc.dram_tensor("y", [64, T, D * SH * W], fp32, kind="ExternalOutput")

    with tile.TileContext(nc) as tc:
        with (
            tc.tile_pool(name="xw", bufs=1) as xw,
            tc.tile_pool(name="hp", bufs=1) as hpool,
            tc.tile_pool(name="st", bufs=2) as stp,
            tc.tile_pool(name="p1", bufs=4, space="PSUM") as p1,
            tc.tile_pool(name="p2", bufs=4, space="PSUM") as p2,
        ):
            w1p = xw.tile([128, 27, 128], fp16)
            nc.sync.dma_start(w1p[:, :, :], w1p_d.ap())
            w1pb = xw.tile([128, 9, 128], fp16)
            nc.sync.dma_start(w1pb[:, :, :], w1pb_d.ap())
            w1s = xw.tile([128, 9, 128], fp16)
            nc.sync.dma_start(w1s[:, :, :], w1s_d.ap())
            w2 = xw.tile([128, 81, 64], fp16)
            nc.sync.dma_start(w2[:, :, :], w2_d.ap())
            b1 = xw.tile([128, 1], fp32)
            nc.sync.dma_start(b1[:, :], b1_d.ap())
            b2 = xw.tile([64, 1], fp32)
            nc.sync.dma_start(b2[:, :], b2_d.ap())
            mt = xw.tile([128, 1], fp32)
            nc.sync.dma_start(mt[:, :], mt_d.ap())
            mb = xw.tile([128, 1], fp32)
            nc.sync.dma_start(mb[:, :], mb_d.ap())

            # tileA = (x, x+1), tileB = (x+2, x+36): shifted copies so each
            # K=128 matmul contracts two taps; chunked so conv1 starts early
            xa = xw.tile([128, T, XP], fp16)
            xb = xw.tile([128, T, XP], fp16)
            hxp = XP // 2
            for t in range(T):
                for lo, hi in ((0, hxp), (hxp, XP)):
                    for tdst, p0, s in ((xa, 0, 0), (xa, 64, 1),
                                        (xb, 0, 2), (xb, 64, 36)):
                        he = min(hi, XP - s)
                        nc.sync.dma_start(tdst[p0:p0 + 64, t, lo:he],
                                          x_d.ap()[:, t, lo + s:he + s])

            ht = hpool.tile([128, T, HD, HHH, HW_], fp16)
            for t in range(T):
                nc.vector.memset(ht[:, t, :, :, :], 0.0)

            # ---- conv1 ----
            # per valid (kt, ku) block: 4 K=128 pairs + 1 K=64 single:
            #   tileA pairs at q=Bq+kv*34 cover (kv,kw=0)+(kv,kw=1)
            #   tileB pair  at q=Bq       covers (0,2)+(1,2)
            #   tileB-top single at q=Bq+68 covers (2,2)
            for t in range(T):
                for d in range(D):
                    blocks = [(kt, ku) for kt in _t_taps(t)
                              for ku in range(3) if 0 <= d + ku - 1 < D]
                    ps = p1.tile([128, HHH, XROW], fp32)
                    nmm = len(blocks) * 5
                    i = 0
                    for kt, ku in blocks:
                        tp = t + kt - 1
                        g9 = kt * 3 + ku
                        bq = (d + ku - 1) * XDP
                        for kv in range(3):
                            nc.tensor.matmul(
                                ps[:, :, :], w1p[:, _g27(kt, ku, kv), :],
                                xa[:, tp, bq + kv * XROW:bq + kv * XROW + N1],
                                start=(i == 0), stop=False)
                            i += 1
                        nc.tensor.matmul(
                            ps[:, :, :], w1pb[:, g9, :],
                            xb[:, tp, bq:bq + N1],
                            start=False, stop=False)
                        i += 1
                        nc.tensor.matmul(
                            ps[:, :, :], w1s[0:64, g9, :],
                            xb[0:64, tp, bq + 68:bq + 68 + N1],
                            start=False, stop=(i == nmm - 1))
                        i += 1
                    nc.scalar.activation(
                        ht[:, t, d + 1, :, 1:33], ps[:, :, 1:33],
                        mybir.ActivationFunctionType.Relu, bias=b1[:, 0:1])
                # zero out-of-image h halo rows (mask is 0 only on edge cores)
                nc.vector.tensor_scalar_mul(
                    ht[:, t, :, 0, 1:33], ht[:, t, :, 0, 1:33], mt[:, 0:1])
                nc.vector.tensor_scalar_mul(
                    ht[:, t, :, HHH - 1, 1:33], ht[:, t, :, HHH - 1, 1:33],
                    mb[:, 0:1])

            # ---- conv2 ----
            # taps alternate between PE column groups (psum partitions 0:64 /
            # 64:128) so adjacent matmuls run concurrently; halves summed on DVE
            for t in range(T):
                taps = [(kt, ku, kv, kw) for kt in _t_taps(t)
                        for ku in range(3) for kv in range(3) for kw in range(3)]
                st = stp.tile([64, D // 2, N2], fp32)
                lo = taps[0::2]
                hi = taps[1::2]
                for dp in range(D // 2):
                    d0 = 2 * dp
                    if PAIR_CONV2:
                        ps = p2.tile([128, N2], fp32)
                        for i in range(len(lo)):
                            for half, base, tp_pos in ((lo, 0, (0, 0)),
                                                       (hi, 64, (0, 64))):
                                if i >= len(half):
                                    continue
                                kt, ku, kv, kw = half[i]
                                gi = _g81(kt, ku, kv, kw)
                                rhs = ht[:, t + kt - 1, d0 + ku:d0 + ku + 2,
                                         kv:kv + SH, kw:kw + W]
                                nc.tensor.matmul(
                                    ps[base:base + 64, :], w2[:, gi, :], rhs,
                                    start=(i == 0), stop=(i == len(half) - 1),
                                    tile_position=tp_pos)
                        nc.scalar.activation(
                            st[:, dp, :], ps[64:128, :],
                            mybir.ActivationFunctionType.Identity, bias=b2[:, 0:1])
                        nc.vector.tensor_add(st[:, dp, :], st[:, dp, :],
                                             ps[0:64, :])
                    else:
                        ps = p2.tile([64, N2], fp32)
                        n = len(taps)
                        for i, (kt, ku, kv, kw) in enumerate(taps):
                            gi = _g81(kt, ku, kv, kw)
                            rhs = ht[:, t + kt - 1, d0 + ku:d0 + ku + 2,
                                     kv:kv + SH, kw:kw + W]
                            nc.tensor.matmul(ps[:, :], w2[:, gi, :], rhs,
                                             start=(i == 0), stop=(i == n - 1))
                        nc.vector.tensor_scalar_add(st[:, dp, :], ps[:, :],
                                                    b2[:, 0:1])
                nc.sync.dma_start(y_d.ap()[:, t, :], st[:, :, :])
    nc.compile()
    return nc


def kernel(x, w1, b1, w2, b2):
    from concourse.bass_utils import run_bass_kernel_spmd

    hostd = _make_host_arrays(x, w1, b1, w2, b2)
    if "nc" not in _cache:
        _cache["nc"] = _build_module()
    nc = _cache["nc"]

    in_maps = []
    for core in range(NCORES):
        in_maps.append({
            "x": hostd["X"][core], "mt": hostd["MT"][core],
            "mb": hostd["MB"][core],
            "w1p": hostd["W1P"], "w1pb": hostd["W1PB"],
            "w1s": hostd["W1S"], "w2": hostd["W2"],
            "b1": hostd["B1"], "b2": hostd["B2"],
        })
    res = run_bass_kernel_spmd(nc, in_maps, core_ids=list(range(NCORES)))

    y = np.zeros((B, C_OUT, T, D, H, W), np.float32)
    for core in range(NCORES):
        b, j = divmod(core, NJ)
        yc = res.results[core]["y"].reshape(C_OUT, T, D, SH, W)
        y[b, :, :, :, SH * j:SH * (j + 1), :] = yc
    return y


# revision 11
# speedup vs baseline: 1.1361x; 1.1361x over previous
"""4D Conv-MLP (conv3^4 -> ReLU -> conv3^4) on 8 Trainium2 NeuronCores.

Sharding: core = b*4 + j  (batch b in {0,1}, H-slab j in {0..3}, 8 output rows
each). Each core computes its output slab independently: conv1 is recomputed on
a 1-row h halo (10 h rows from 12 x rows), so no cross-core communication is
needed. One SPMD program for all cores; per-core boundary behavior is driven by
data (host-zeroed x halos + h halo-row masks).

On-chip algorithm (implicit GEMM over the 81 taps, fp16 operands, fp32 PSUM):
  - x is stored channel-on-partition as a zero-padded flat plane per t:
    [18 D][12 H][34 W] (+1 lead pad), duplicated on partitions 64..127 shifted
    by one element so each K=128 matmul contracts two W-taps at once.
  - conv1: per (t, d): 340-column matmuls accumulating (kt,ku,kv) pair+single
    taps; ReLU+bias on the Scalar engine writes fp16 h (pads skipped).
  - conv2: per (t, d-pair): 512-column matmuls over all valid taps with
    K=128; bias added on DVE; fp32 result DMAd out.
"""

import numpy as np

B, C_IN, C_HID, C_OUT = 2, 64, 128, 64
T, D, H, W = 4, 16, 32, 32
NCORES, NJ = 8, 4
SH = H // NJ          # 8 out rows per slab
XH = SH + 4           # 12 x rows per slab
HHH = SH + 2          # 10 h rows per slab
XROW = 34             # padded W
XDP = 12 * XROW       # 408
XP = 1 + 16 * XDP + 7   # x plane size (real D rows only) = 6536
HD, HW_ = 18, 34
HP = HD * HHH * HW_   # h plane = 6120
N1 = HHH * XROW       # conv1 run = 340
N2 = 512              # conv2 run (2 d-rows)

_cache = {}
import os
PAIR_CONV2 = os.environ.get('K_PAIR_CONV2', '1') == '1'
PAIR_SINGLES = os.environ.get('K_PAIR_SINGLES', '1') == '1'


def _t_taps(t):
    return [kt for kt in range(3) if 0 <= t + kt - 1 < T]


def _g27(kt, ku, kv):
    return (kt * 3 + ku) * 3 + kv


def _g81(kt, ku, kv, kw):
    return ((kt * 3 + ku) * 3 + kv) * 3 + kw


def _make_host_arrays(x, w1, b1, w2, b2):
    x = np.asarray(x, np.float32)
    Xs, MTs, MBs = [], [], []
    for core in range(NCORES):
        b, j = divmod(core, NJ)
        h0 = SH * j
        slab = np.zeros((C_IN, T, D, XH, W), np.float32)
        lo, hi = h0 - 2, h0 + 10
        slo, shi = max(lo, 0), min(hi, H)
        slab[:, :, :, slo - lo:shi - lo, :] = x[b, :, :, :, slo:shi, :]
        plane = np.zeros((C_IN, T, D, XH, XROW), np.float32)
        plane[:, :, :, :, 1:33] = slab
        flat = plane.reshape(C_IN, T, D * XDP)
        X = np.zeros((C_IN, T, XP), np.float16)
        X[:, :, 1:1 + D * XDP] = flat
        Xs.append(X)
        MTs.append(np.full((128, 1), 0.0 if j == 0 else 1.0, np.float32))
        MBs.append(np.full((128, 1), 0.0 if j == NJ - 1 else 1.0, np.float32))

    w1 = np.asarray(w1, np.float32)
    w2 = np.asarray(w2, np.float32)
    W1P = np.zeros((128, 27, 128), np.float16)   # tileA pairs: (kv,kw=0)+(kv,kw=1)
    W1PB = np.zeros((128, 9, 128), np.float16)   # tileB pair: (0,2)+(1,2)
    W1S = np.zeros((128, 9, 128), np.float16)    # tileB-top single: (2,2)
    for kt in range(3):
        for ku in range(3):
            g9 = kt * 3 + ku
            W1PB[:64, g9, :] = w1[:, :, kt, ku, 0, 2].T
            W1PB[64:, g9, :] = w1[:, :, kt, ku, 1, 2].T
            W1S[:64, g9, :] = w1[:, :, kt, ku, 2, 2].T
            for kv in range(3):
                g = _g27(kt, ku, kv)
                W1P[:64, g, :] = w1[:, :, kt, ku, kv, 0].T
                W1P[64:, g, :] = w1[:, :, kt, ku, kv, 1].T
    W2 = np.zeros((128, 81, 64), np.float16)
    for kt in range(3):
        for ku in range(3):
            for kv in range(3):
                for kw in range(3):
                    gi = _g81(kt, ku, kv, kw)
                    W2[:, gi, :] = w2[:, :, kt, ku, kv, kw].T
    return dict(X=Xs, MT=MTs, MB=MBs,
                W1P=W1P.reshape(128, 27 * 128), W1PB=W1PB.reshape(128, 9 * 128),
                W1S=W1S.reshape(128, 9 * 128),
                W2=W2.reshape(128, 81 * 64),
                B1=np.asarray(b1, np.float32).reshape(128, 1),
                B2=np.asarray(b2, np.float32).reshape(64, 1))


def _build_module():
    import concourse.bass as bass
    import concourse.tile as tile
    from concourse import bacc, mybir

    fp16 = mybir.dt.float16
    fp32 = mybir.dt.float32

    nc = bacc.Bacc("TRN2", target_bir_lowering=False, debug=False, num_devices=1)
    x_d = nc.dram_tensor("x", [64, T, XP], fp16, kind="ExternalInput")
    w1p_d = nc.dram_tensor("w1p", [128, 27 * 128], fp16, kind="ExternalInput")
    w1pb_d = nc.dram_tensor("w1pb", [128, 9 * 128], fp16, kind="ExternalInput")
    w1s_d = nc.dram_tensor("w1s", [128, 9 * 128], fp16, kind="ExternalInput")
    w2_d = nc.dram_tensor("w2", [128, 81 * 64], fp16, kind="ExternalInput")
    b1_d = nc.dram_tensor("b1", [128, 1], fp32, kind="ExternalInput")
    b2_d = nc.dram_tensor("b2", [64, 1], fp32, kind="ExternalInput")
    mt_d = nc.dram_tensor("mt", [128, 1], fp32, kind="ExternalInput")
    mb_d = nc.dram_tensor("mb", [128, 1], fp32, kind="ExternalInput")
    y_d = nc.dram_tensor("y", [64, T, D * SH * W], fp32, kind="ExternalOutput")

    with tile.TileContext(nc) as tc:
        with (
            tc.tile_pool(name="xw", bufs=1) as xw,
            tc.tile_pool(name="hp", bufs=1) as hpool,
            tc.tile_pool(name="st", bufs=2) as stp,
            tc.tile_pool(name="p1", bufs=4, space="PSUM") as p1,
            tc.tile_pool(name="p2", bufs=4, space="PSUM") as p2,
        ):
            w1p = xw.tile([128, 27, 128], fp16)
            nc.sync.dma_start(w1p[:, :, :], w1p_d.ap())
            w1pb = xw.tile([128, 9, 128], fp16)
            nc.sync.dma_start(w1pb[:, :, :], w1pb_d.ap())
            w1s = xw.tile([128, 9, 128], fp16)
            nc.sync.dma_start(w1s[:, :, :], w1s_d.ap())
            w2 = xw.tile([128, 81, 64], fp16)
            nc.sync.dma_start(w2[:, :, :], w2_d.ap())
            b1 = xw.tile([128, 1], fp32)
            nc.sync.dma_start(b1[:, :], b1_d.ap())
            b2 = xw.tile([64, 1], fp32)
            nc.sync.dma_start(b2[:, :], b2_d.ap())
            mt = xw.tile([128, 1], fp32)
            nc.sync.dma_start(mt[:, :], mt_d.ap())
            mb = xw.tile([128, 1], fp32)
            nc.sync.dma_start(mb[:, :], mb_d.ap())

            # tileA = (x, x+1), tileB = (x+2, x+36): shifted copies so each
            # K=128 matmul contracts two taps; chunked so conv1 starts early
            xa = xw.tile([128, T, XP], fp16)
            xb = xw.tile([128, T, XP], fp16)
            hxp = XP // 2
            for t in range(T):
                for lo, hi in ((0, hxp), (hxp, XP)):
                    for tdst, p0, s in ((xa, 0, 0), (xa, 64, 1),
                                        (xb, 0, 2), (xb, 64, 36)):
                        he = min(hi, XP - s)
                        nc.sync.dma_start(tdst[p0:p0 + 64, t, lo:he],
                                          x_d.ap()[:, t, lo + s:he + s])

            ht = hpool.tile([128, T, HD, HHH, HW_], fp16)
            for t in range(T):
                nc.vector.memset(ht[:, t, :, :, :], 0.0)

            # ---- conv1 ----
            # per valid (kt, ku) block: 4 K=128 pairs + 1 K=64 single:
            #   tileA pairs at q=Bq+kv*34 cover (kv,kw=0)+(kv,kw=1)
            #   tileB pair  at q=Bq       covers (0,2)+(1,2)
            #   tileB-top single at q=Bq+68 covers (2,2)
            for t in range(T):
                for d in range(D):
                    blocks = [(kt, ku) for kt in _t_taps(t)
                              for ku in range(3) if 0 <= d + ku - 1 < D]
                    ps = p1.tile([128, HHH, XROW], fp32)
                    # all K=128 matmuls first, then all K=64 singles, so the
                    # PE sees only one tile_size transition per run (tile
                    # switches stall the LDWEIGHTS pipeline)
                    i = 0
                    for kt, ku in blocks:
                        tp = t + kt - 1
                        bq = (d + ku - 1) * XDP
                        for kv in range(3):
                            nc.tensor.matmul(
                                ps[:, :, :], w1p[:, _g27(kt, ku, kv), :],
                                xa[:, tp, bq + kv * XROW:bq + kv * XROW + N1],
                                start=(i == 0), stop=False)
                            i += 1
                        nc.tensor.matmul(
                            ps[:, :, :], w1pb[:, kt * 3 + ku, :],
                            xb[:, tp, bq:bq + N1],
                            start=False, stop=False)
                        i += 1
                    for i, (kt, ku) in enumerate(blocks):
                        tp = t + kt - 1
                        bq = (d + ku - 1) * XDP
                        nc.tensor.matmul(
                            ps[:, :, :], w1s[0:64, kt * 3 + ku, :],
                            xb[0:64, tp, bq + 68:bq + 68 + N1],
                            start=False, stop=(i == len(blocks) - 1))
                    nc.scalar.activation(
                        ht[:, t, d + 1, :, 1:33], ps[:, :, 1:33],
                        mybir.ActivationFunctionType.Relu, bias=b1[:, 0:1])
                # zero out-of-image h halo rows (mask is 0 only on edge cores)
                nc.vector.tensor_scalar_mul(
                    ht[:, t, :, 0, 1:33], ht[:, t, :, 0, 1:33], mt[:, 0:1])
                nc.vector.tensor_scalar_mul(
                    ht[:, t, :, HHH - 1, 1:33], ht[:, t, :, HHH - 1, 1:33],
                    mb[:, 0:1])

            # ---- conv2 ----
            # taps alternate between PE column groups (psum partitions 0:64 /
            # 64:128) so adjacent matmuls run concurrently; halves summed on DVE
            for t in range(T):
                taps = [(kt, ku, kv, kw) for kt in _t_taps(t)
                        for ku in range(3) for kv in range(3) for kw in range(3)]
                st = stp.tile([64, D // 2, N2], fp32)
                lo = taps[0::2]
                hi = taps[1::2]
                for dp in range(D // 2):
                    d0 = 2 * dp
                    if PAIR_CONV2:
                        ps = p2.tile([128, N2], fp32)
                        for i in range(len(lo)):
                            for half, base, tp_pos in ((lo, 0, (0, 0)),
                                                       (hi, 64, (0, 64))):
                                if i >= len(half):
                                    continue
                                kt, ku, kv, kw = half[i]
                                gi = _g81(kt, ku, kv, kw)
                                rhs = ht[:, t + kt - 1, d0 + ku:d0 + ku + 2,
                                         kv:kv + SH, kw:kw + W]
                                nc.tensor.matmul(
                                    ps[base:base + 64, :], w2[:, gi, :], rhs,
                                    start=(i == 0), stop=(i == len(half) - 1),
                                    tile_position=tp_pos)
                        nc.scalar.activation(
                            st[:, dp, :], ps[64:128, :],
                            mybir.ActivationFunctionType.Identity, bias=b2[:, 0:1])
                        nc.vector.tensor_add(st[:, dp, :], st[:, dp, :],
                                             ps[0:64, :])
                    else:
                        ps = p2.tile([64, N2], fp32)
                        n = len(taps)
                        for i, (kt, ku, kv, kw) in enumerate(taps):
                            gi = _g81(kt, ku, kv, kw)
                            rhs = ht[:, t + kt - 1, d0 + ku:d0 + ku + 2,
                                     kv:kv + SH, kw:kw + W]
                            nc.tensor.matmul(ps[:, :], w2[:, gi, :], rhs,
                                             start=(i == 0), stop=(i == n - 1))
                        nc.vector.tensor_scalar_add(st[:, dp, :], ps[:, :],
                                                    b2[:, 0:1])
                nc.sync.dma_start(y_d.ap()[:, t, :], st[:, :, :])
    nc.compile()
    return nc


def kernel(x, w1, b1, w2, b2):
    from concourse.bass_utils import run_bass_kernel_spmd

    hostd = _make_host_arrays(x, w1, b1, w2, b2)
    if "nc" not in _cache:
        _cache["nc"] = _build_module()
    nc = _cache["nc"]

    in_maps = []
    for core in range(NCORES):
        in_maps.append({
            "x": hostd["X"][core], "mt": hostd["MT"][core],
            "mb": hostd["MB"][core],
            "w1p": hostd["W1P"], "w1pb": hostd["W1PB"],
            "w1s": hostd["W1S"], "w2": hostd["W2"],
            "b1": hostd["B1"], "b2": hostd["B2"],
        })
    res = run_bass_kernel_spmd(nc, in_maps, core_ids=list(range(NCORES)))

    y = np.zeros((B, C_OUT, T, D, H, W), np.float32)
    for core in range(NCORES):
        b, j = divmod(core, NJ)
        yc = res.results[core]["y"].reshape(C_OUT, T, D, SH, W)
        y[b, :, :, :, SH * j:SH * (j + 1), :] = yc
    return y


# revision 12
# speedup vs baseline: 1.1726x; 1.0321x over previous
"""4D Conv-MLP (conv3^4 -> ReLU -> conv3^4) on 8 Trainium2 NeuronCores.

Sharding: core = b*4 + j  (batch b in {0,1}, H-slab j in {0..3}, 8 output rows
each). Each core computes its output slab independently: conv1 is recomputed on
a 1-row h halo (10 h rows from 12 x rows), so no cross-core communication is
needed. One SPMD program for all cores; per-core boundary behavior is driven by
data (host-zeroed x halos + h halo-row masks).

On-chip algorithm (implicit GEMM over the 81 taps, fp16 operands, fp32 PSUM):
  - x is stored channel-on-partition as a zero-padded flat plane per t:
    [18 D][12 H][34 W] (+1 lead pad), duplicated on partitions 64..127 shifted
    by one element so each K=128 matmul contracts two W-taps at once.
  - conv1: per (t, d): 340-column matmuls accumulating (kt,ku,kv) pair+single
    taps; ReLU+bias on the Scalar engine writes fp16 h (pads skipped).
  - conv2: per (t, d-pair): 512-column matmuls over all valid taps with
    K=128; bias added on DVE; fp32 result DMAd out.
"""

import numpy as np

B, C_IN, C_HID, C_OUT = 2, 64, 128, 64
T, D, H, W = 4, 16, 32, 32
NCORES, NJ = 8, 4
SH = H // NJ          # 8 out rows per slab
XH = SH + 4           # 12 x rows per slab
HHH = SH + 2          # 10 h rows per slab
XROW = 34             # padded W
XDP = 12 * XROW       # 408
XP = 1 + 16 * XDP + 7   # x plane size (real D rows only) = 6536
HD, HW_ = 18, 34
HP = HD * HHH * HW_   # h plane = 6120
N1 = HHH * XROW       # conv1 run = 340
N2 = 512              # conv2 run (2 d-rows)

_cache = {}
import os
PAIR_CONV2 = os.environ.get('K_PAIR_CONV2', '1') == '1'
PAIR_SINGLES = os.environ.get('K_PAIR_SINGLES', '1') == '1'


def _t_taps(t):
    return [kt for kt in range(3) if 0 <= t + kt - 1 < T]


def _g27(kt, ku, kv):
    return (kt * 3 + ku) * 3 + kv


def _g81(kt, ku, kv, kw):
    return ((kt * 3 + ku) * 3 + kv) * 3 + kw


def _make_host_arrays(x, w1, b1, w2, b2):
    x = np.asarray(x, np.float32)
    Xs, MTs, MBs = [], [], []
    for core in range(NCORES):
        b, j = divmod(core, NJ)
        h0 = SH * j
        slab = np.zeros((C_IN, T, D, XH, W), np.float32)
        lo, hi = h0 - 2, h0 + 10
        slo, shi = max(lo, 0), min(hi, H)
        slab[:, :, :, slo - lo:shi - lo, :] = x[b, :, :, :, slo:shi, :]
        plane = np.zeros((C_IN, T, D, XH, XROW), np.float32)
        plane[:, :, :, :, 1:33] = slab
        flat = plane.reshape(C_IN, T, D * XDP)
        X = np.zeros((C_IN, T, XP), np.float16)
        X[:, :, 1:1 + D * XDP] = flat
        Xs.append(X)
        MTs.append(np.full((128, 1), 0.0 if j == 0 else 1.0, np.float32))
        MBs.append(np.full((128, 1), 0.0 if j == NJ - 1 else 1.0, np.float32))

    w1 = np.asarray(w1, np.float32)
    w2 = np.asarray(w2, np.float32)
    W1P = np.zeros((128, 27, 128), np.float16)   # tileA pairs: (kv,kw=0)+(kv,kw=1)
    W1PB = np.zeros((128, 9, 128), np.float16)   # tileB pair: (0,2)+(1,2)
    W1S = np.zeros((128, 9, 128), np.float16)    # tileB-top single: (2,2)
    for kt in range(3):
        for ku in range(3):
            g9 = kt * 3 + ku
            W1PB[:64, g9, :] = w1[:, :, kt, ku, 0, 2].T
            W1PB[64:, g9, :] = w1[:, :, kt, ku, 1, 2].T
            W1S[:64, g9, :] = w1[:, :, kt, ku, 2, 2].T
            for kv in range(3):
                g = _g27(kt, ku, kv)
                W1P[:64, g, :] = w1[:, :, kt, ku, kv, 0].T
                W1P[64:, g, :] = w1[:, :, kt, ku, kv, 1].T
    W2 = np.zeros((128, 81, 64), np.float16)
    for kt in range(3):
        for ku in range(3):
            for kv in range(3):
                for kw in range(3):
                    gi = _g81(kt, ku, kv, kw)
                    W2[:, gi, :] = w2[:, :, kt, ku, kv, kw].T
    return dict(X=Xs, MT=MTs, MB=MBs,
                W1P=W1P.reshape(128, 27 * 128), W1PB=W1PB.reshape(128, 9 * 128),
                W1S=W1S.reshape(128, 9 * 128),
                W2=W2.reshape(128, 81 * 64),
                B1=np.asarray(b1, np.float32).reshape(128, 1),
                B2=np.asarray(b2, np.float32).reshape(64, 1))


def _build_module():
    import concourse.bass as bass
    import concourse.tile as tile
    from concourse import bacc, mybir

    fp16 = mybir.dt.float16
    fp32 = mybir.dt.float32

    nc = bacc.Bacc("TRN2", target_bir_lowering=False, debug=False, num_devices=1)
    x_d = nc.dram_tensor("x", [64, T, XP], fp16, kind="ExternalInput")
    w1p_d = nc.dram_tensor("w1p", [128, 27 * 128], fp16, kind="ExternalInput")
    w1pb_d = nc.dram_tensor("w1pb", [128, 9 * 128], fp16, kind="ExternalInput")
    w1s_d = nc.dram_tensor("w1s", [128, 9 * 128], fp16, kind="ExternalInput")
    w2_d = nc.dram_tensor("w2", [128, 81 * 64], fp16, kind="ExternalInput")
    b1_d = nc.dram_tensor("b1", [128, 1], fp32, kind="ExternalInput")
    b2_d = nc.dram_tensor("b2", [64, 1], fp32, kind="ExternalInput")
    mt_d = nc.dram_tensor("mt", [128, 1], fp32, kind="ExternalInput")
    mb_d = nc.dram_tensor("mb", [128, 1], fp32, kind="ExternalInput")
    y_d = nc.dram_tensor("y", [64, T, D * SH * W], fp32, kind="ExternalOutput")

    with tile.TileContext(nc) as tc:
        with (
            tc.tile_pool(name="xw", bufs=1) as xw,
            tc.tile_pool(name="hp", bufs=1) as hpool,
            tc.tile_pool(name="st", bufs=4) as stp,
            tc.tile_pool(name="p1", bufs=4, space="PSUM") as p1,
            tc.tile_pool(name="p2", bufs=4, space="PSUM") as p2,
        ):
            w1p = xw.tile([128, 27, 128], fp16)
            nc.sync.dma_start(w1p[:, :, :], w1p_d.ap())
            b1 = xw.tile([128, 1], fp32)
            nc.sync.dma_start(b1[:, :], b1_d.ap())

            # tileA = (x, x+1), tileB = (x+2, x+36): shifted copies so each
            # K=128 matmul contracts two taps; quarter-chunked, first chunks
            # DMAd first so conv1 can start early
            xa = xw.tile([128, T, XP], fp16)
            xb = xw.tile([128, T, XP], fp16)
            qs = [0, XP // 4, XP // 2, 3 * XP // 4, XP]
            for ci in range(4):
                lo, hi = qs[ci], qs[ci + 1]
                for t in range(T):
                    for tdst, p0, s in ((xa, 0, 0), (xa, 64, 1),
                                        (xb, 0, 2), (xb, 64, 36)):
                        he = min(hi, XP - s)
                        nc.sync.dma_start(tdst[p0:p0 + 64, t, lo:he],
                                          x_d.ap()[:, t, lo + s:he + s])
                if ci == 0:
                    w1pb = xw.tile([128, 9, 128], fp16)
                    nc.sync.dma_start(w1pb[:, :, :], w1pb_d.ap())
                    w1s = xw.tile([128, 9, 128], fp16)
                    nc.sync.dma_start(w1s[:, :, :], w1s_d.ap())

            w2 = xw.tile([128, 81, 64], fp16)
            nc.sync.dma_start(w2[:, :, :], w2_d.ap())
            b2 = xw.tile([64, 1], fp32)
            nc.sync.dma_start(b2[:, :], b2_d.ap())
            mt = xw.tile([128, 1], fp32)
            nc.sync.dma_start(mt[:, :], mt_d.ap())
            mb = xw.tile([128, 1], fp32)
            nc.sync.dma_start(mb[:, :], mb_d.ap())

            ht = hpool.tile([128, T, HD, HHH, HW_], fp16)
            for t in range(T):
                nc.vector.memset(ht[:, t, :, :, :], 0.0)

            # ---- conv1 ----
            # per valid (kt, ku) block: 4 K=128 pairs + 1 K=64 single:
            #   tileA pairs at q=Bq+kv*34 cover (kv,kw=0)+(kv,kw=1)
            #   tileB pair  at q=Bq       covers (0,2)+(1,2)
            #   tileB-top single at q=Bq+68 covers (2,2)
            for t in range(T):
                for d in range(D):
                    blocks = [(kt, ku) for kt in _t_taps(t)
                              for ku in range(3) if 0 <= d + ku - 1 < D]
                    ps = p1.tile([128, HHH, XROW], fp32)
                    # all K=128 matmuls first, then all K=64 singles, so the
                    # PE sees only one tile_size transition per run (tile
                    # switches stall the LDWEIGHTS pipeline)
                    i = 0
                    for kt, ku in blocks:
                        tp = t + kt - 1
                        bq = (d + ku - 1) * XDP
                        for kv in range(3):
                            nc.tensor.matmul(
                                ps[:, :, :], w1p[:, _g27(kt, ku, kv), :],
                                xa[:, tp, bq + kv * XROW:bq + kv * XROW + N1],
                                start=(i == 0), stop=False)
                            i += 1
                        nc.tensor.matmul(
                            ps[:, :, :], w1pb[:, kt * 3 + ku, :],
                            xb[:, tp, bq:bq + N1],
                            start=False, stop=False)
                        i += 1
                    for i, (kt, ku) in enumerate(blocks):
                        tp = t + kt - 1
                        bq = (d + ku - 1) * XDP
                        nc.tensor.matmul(
                            ps[:, :, :], w1s[0:64, kt * 3 + ku, :],
                            xb[0:64, tp, bq + 68:bq + 68 + N1],
                            start=False, stop=(i == len(blocks) - 1))
                    nc.scalar.activation(
                        ht[:, t, d + 1, :, 1:33], ps[:, :, 1:33],
                        mybir.ActivationFunctionType.Relu, bias=b1[:, 0:1])
                # zero out-of-image h halo rows (mask is 0 only on edge cores)
                nc.vector.tensor_scalar_mul(
                    ht[:, t, :, 0, 1:33], ht[:, t, :, 0, 1:33], mt[:, 0:1])
                nc.vector.tensor_scalar_mul(
                    ht[:, t, :, HHH - 1, 1:33], ht[:, t, :, HHH - 1, 1:33],
                    mb[:, 0:1])

            # ---- conv2 ----
            # runs: edge d=0 and d=15 alone (N=256, zero-pad taps skipped),
            # interior d as 7 pairs (N=512). Taps alternate between PE column
            # groups (psum partitions 0:64 / 64:128) so adjacent matmuls run
            # concurrently; halves summed via Scalar+DVE into the stage tile.
            runs = [(0, 1)] + [(d0, 2) for d0 in range(1, 15, 2)] + [(15, 1)]
            for t in range(T):
                for d0, nd in runs:
                    taps = [(kt, ku, kv, kw) for kt in _t_taps(t)
                            for ku in range(3) if 0 < d0 + ku < 17 or nd == 2
                            for kv in range(3) for kw in range(3)]
                    nn = nd * SH * W
                    lo = taps[0::2]
                    hi = taps[1::2]
                    ps = p2.tile([128, N2], fp32)
                    for i in range(len(lo)):
                        for half, base, tp_pos in ((lo, 0, (0, 0)),
                                                   (hi, 64, (0, 64))):
                            if i >= len(half):
                                continue
                            kt, ku, kv, kw = half[i]
                            gi = _g81(kt, ku, kv, kw)
                            rhs = ht[:, t + kt - 1, d0 + ku:d0 + ku + nd,
                                     kv:kv + SH, kw:kw + W]
                            nc.tensor.matmul(
                                ps[base:base + 64, 0:nn], w2[:, gi, :], rhs,
                                start=(i == 0), stop=(i == len(half) - 1),
                                tile_position=tp_pos)
                    st = stp.tile([64, N2], fp32)
                    nc.scalar.activation(
                        st[:, 0:nn], ps[64:128, 0:nn],
                        mybir.ActivationFunctionType.Identity, bias=b2[:, 0:1])
                    nc.vector.tensor_add(st[:, 0:nn], st[:, 0:nn],
                                         ps[0:64, 0:nn])
                    nc.sync.dma_start(
                        y_d.ap()[:, t, d0 * SH * W:d0 * SH * W + nn],
                        st[:, 0:nn])
    nc.compile()
    return nc


def kernel(x, w1, b1, w2, b2):
    from concourse.bass_utils import run_bass_kernel_spmd

    hostd = _make_host_arrays(x, w1, b1, w2, b2)
    if "nc" not in _cache:
        _cache["nc"] = _build_module()
    nc = _cache["nc"]

    in_maps = []
    for core in range(NCORES):
        in_maps.append({
            "x": hostd["X"][core], "mt": hostd["MT"][core],
            "mb": hostd["MB"][core],
            "w1p": hostd["W1P"], "w1pb": hostd["W1PB"],
            "w1s": hostd["W1S"], "w2": hostd["W2"],
            "b1": hostd["B1"], "b2": hostd["B2"],
        })
    res = run_bass_kernel_spmd(nc, in_maps, core_ids=list(range(NCORES)))

    y = np.zeros((B, C_OUT, T, D, H, W), np.float32)
    for core in range(NCORES):
        b, j = divmod(core, NJ)
        yc = res.results[core]["y"].reshape(C_OUT, T, D, SH, W)
        y[b, :, :, :, SH * j:SH * (j + 1), :] = yc
    return y


# revision 13
# speedup vs baseline: 1.1880x; 1.0132x over previous
"""4D Conv-MLP (conv3^4 -> ReLU -> conv3^4) on 8 Trainium2 NeuronCores.

Sharding: core = b*4 + j  (batch b in {0,1}, H-slab j in {0..3}, 8 output rows
each). Each core computes its output slab independently: conv1 is recomputed on
a 1-row h halo (10 h rows from 12 x rows), so no cross-core communication is
needed. One SPMD program for all cores; per-core boundary behavior is driven by
data (host-zeroed x halos + h halo-row masks).

On-chip algorithm (implicit GEMM over the 81 taps, fp16 operands, fp32 PSUM):
  - x is stored channel-on-partition as a zero-padded flat plane per t:
    [18 D][12 H][34 W] (+1 lead pad), duplicated on partitions 64..127 shifted
    by one element so each K=128 matmul contracts two W-taps at once.
  - conv1: per (t, d): 340-column matmuls accumulating (kt,ku,kv) pair+single
    taps; ReLU+bias on the Scalar engine writes fp16 h (pads skipped).
  - conv2: per (t, d-pair): 512-column matmuls over all valid taps with
    K=128; bias added on DVE; fp32 result DMAd out.
"""

import numpy as np

B, C_IN, C_HID, C_OUT = 2, 64, 128, 64
T, D, H, W = 4, 16, 32, 32
NCORES, NJ = 8, 4
SH = H // NJ          # 8 out rows per slab
XH = SH + 4           # 12 x rows per slab
HHH = SH + 2          # 10 h rows per slab
XROW = 34             # padded W
XDP = 12 * XROW       # 408
XP = 1 + 16 * XDP + 7   # x plane size (real D rows only) = 6536
HD, HW_ = 18, 34
HP = HD * HHH * HW_   # h plane = 6120
N1 = HHH * XROW       # conv1 run = 340
N2 = 512              # conv2 run (2 d-rows)

_cache = {}
import os
PAIR_CONV2 = os.environ.get('K_PAIR_CONV2', '1') == '1'
PAIR_SINGLES = os.environ.get('K_PAIR_SINGLES', '1') == '1'


def _t_taps(t):
    return [kt for kt in range(3) if 0 <= t + kt - 1 < T]


def _g27(kt, ku, kv):
    return (kt * 3 + ku) * 3 + kv


def _g81(kt, ku, kv, kw):
    return ((kt * 3 + ku) * 3 + kv) * 3 + kw


def _make_host_arrays(x, w1, b1, w2, b2):
    x = np.asarray(x, np.float32)
    Xs, MTs, MBs = [], [], []
    for core in range(NCORES):
        b, j = divmod(core, NJ)
        h0 = SH * j
        slab = np.zeros((C_IN, T, D, XH, W), np.float32)
        lo, hi = h0 - 2, h0 + 10
        slo, shi = max(lo, 0), min(hi, H)
        slab[:, :, :, slo - lo:shi - lo, :] = x[b, :, :, :, slo:shi, :]
        plane = np.zeros((C_IN, T, D, XH, XROW), np.float32)
        plane[:, :, :, :, 1:33] = slab
        flat = plane.reshape(C_IN, T, D * XDP)
        X = np.zeros((C_IN, T, XP), np.float16)
        X[:, :, 1:1 + D * XDP] = flat
        Xs.append(X)
        MTs.append(np.full((128, 1), 0.0 if j == 0 else 1.0, np.float32))
        MBs.append(np.full((128, 1), 0.0 if j == NJ - 1 else 1.0, np.float32))

    w1 = np.asarray(w1, np.float32)
    w2 = np.asarray(w2, np.float32)
    W1P = np.zeros((128, 27, 128), np.float16)   # tileA pairs: (kv,kw=0)+(kv,kw=1)
    W1PB = np.zeros((128, 9, 128), np.float16)   # tileB pair: (0,2)+(1,2)
    W1S = np.zeros((128, 9, 128), np.float16)    # tileB-top single: (2,2)
    for kt in range(3):
        for ku in range(3):
            g9 = kt * 3 + ku
            W1PB[:64, g9, :] = w1[:, :, kt, ku, 0, 2].T
            W1PB[64:, g9, :] = w1[:, :, kt, ku, 1, 2].T
            W1S[:64, g9, :] = w1[:, :, kt, ku, 2, 2].T
            for kv in range(3):
                g = _g27(kt, ku, kv)
                W1P[:64, g, :] = w1[:, :, kt, ku, kv, 0].T
                W1P[64:, g, :] = w1[:, :, kt, ku, kv, 1].T
    W2 = np.zeros((128, 81, 64), np.float16)
    for kt in range(3):
        for ku in range(3):
            for kv in range(3):
                for kw in range(3):
                    gi = _g81(kt, ku, kv, kw)
                    W2[:, gi, :] = w2[:, :, kt, ku, kv, kw].T
    return dict(X=Xs, MT=MTs, MB=MBs,
                W1P=W1P.reshape(128, 27 * 128), W1PB=W1PB.reshape(128, 9 * 128),
                W1S=W1S.reshape(128, 9 * 128),
                W2=W2.reshape(128, 81 * 64),
                B1=np.asarray(b1, np.float32).reshape(128, 1),
                B2=np.asarray(b2, np.float32).reshape(64, 1))


def _build_module():
    import concourse.bass as bass
    import concourse.tile as tile
    from concourse import bacc, mybir

    fp16 = mybir.dt.float16
    fp32 = mybir.dt.float32

    nc = bacc.Bacc("TRN2", target_bir_lowering=False, debug=False, num_devices=1)
    x_d = nc.dram_tensor("x", [64, T, XP], fp16, kind="ExternalInput")
    w1p_d = nc.dram_tensor("w1p", [128, 27 * 128], fp16, kind="ExternalInput")
    w1pb_d = nc.dram_tensor("w1pb", [128, 9 * 128], fp16, kind="ExternalInput")
    w1s_d = nc.dram_tensor("w1s", [128, 9 * 128], fp16, kind="ExternalInput")
    w2_d = nc.dram_tensor("w2", [128, 81 * 64], fp16, kind="ExternalInput")
    b1_d = nc.dram_tensor("b1", [128, 1], fp32, kind="ExternalInput")
    b2_d = nc.dram_tensor("b2", [64, 1], fp32, kind="ExternalInput")
    mt_d = nc.dram_tensor("mt", [128, 1], fp32, kind="ExternalInput")
    mb_d = nc.dram_tensor("mb", [128, 1], fp32, kind="ExternalInput")
    y_d = nc.dram_tensor("y", [64, T, D * SH * W], fp32, kind="ExternalOutput")

    with tile.TileContext(nc) as tc:
        with (
            tc.tile_pool(name="xw", bufs=1) as xw,
            tc.tile_pool(name="hp", bufs=1) as hpool,
            tc.tile_pool(name="st", bufs=4) as stp,
            tc.tile_pool(name="p1", bufs=4, space="PSUM") as p1,
            tc.tile_pool(name="p2", bufs=4, space="PSUM") as p2,
        ):
            w1p = xw.tile([128, 27, 128], fp16)
            nc.sync.dma_start(w1p[:, :, :], w1p_d.ap())
            b1 = xw.tile([128, 1], fp32)
            nc.sync.dma_start(b1[:, :], b1_d.ap())

            # tileA = (x, x+1), tileB = (x+2, x+36): shifted copies so each
            # K=128 matmul contracts two taps; quarter-chunked, first chunks
            # DMAd first so conv1 can start early
            xa = xw.tile([128, T, XP], fp16)
            xb = xw.tile([128, T, XP], fp16)
            qs = [0, XP // 4, XP // 2, 3 * XP // 4, XP]

            def xchunk(t, ci):
                lo, hi = qs[ci], qs[ci + 1]
                for tdst, p0, s in ((xa, 0, 0), (xa, 64, 1),
                                    (xb, 0, 2), (xb, 64, 36)):
                    he = min(hi, XP - s)
                    nc.sync.dma_start(tdst[p0:p0 + 64, t, lo:he],
                                      x_d.ap()[:, t, lo + s:he + s])

            # conv1 t=0 needs planes 0,1 and the tileB weights first
            xchunk(0, 0)
            xchunk(1, 0)
            w1pb = xw.tile([128, 9, 128], fp16)
            nc.sync.dma_start(w1pb[:, :, :], w1pb_d.ap())
            w1s = xw.tile([128, 9, 128], fp16)
            nc.sync.dma_start(w1s[:, :, :], w1s_d.ap())
            for t, ci in ((2, 0), (3, 0), (0, 1), (1, 1), (0, 2), (1, 2),
                          (0, 3), (1, 3), (2, 1), (3, 1), (2, 2), (3, 2),
                          (2, 3), (3, 3)):
                xchunk(t, ci)

            w2 = xw.tile([128, 81, 64], fp16)
            nc.sync.dma_start(w2[:, :, :], w2_d.ap())
            b2 = xw.tile([64, 1], fp32)
            nc.sync.dma_start(b2[:, :], b2_d.ap())
            mt = xw.tile([128, 1], fp32)
            nc.sync.dma_start(mt[:, :], mt_d.ap())
            mb = xw.tile([128, 1], fp32)
            nc.sync.dma_start(mb[:, :], mb_d.ap())

            ht = hpool.tile([128, T, HD, HHH, HW_], fp16)
            for t in range(T):
                nc.vector.memset(ht[:, t, :, :, :], 0.0)

            # ---- conv1 ----
            # per valid (kt, ku) block: 4 K=128 pairs + 1 K=64 single:
            #   tileA pairs at q=Bq+kv*34 cover (kv,kw=0)+(kv,kw=1)
            #   tileB pair  at q=Bq       covers (0,2)+(1,2)
            #   tileB-top single at q=Bq+68 covers (2,2)
            for t in range(T):
                for d in range(D):
                    blocks = [(kt, ku) for kt in _t_taps(t)
                              for ku in range(3) if 0 <= d + ku - 1 < D]
                    ps = p1.tile([128, HHH, XROW], fp32)
                    # all K=128 matmuls first, then all K=64 singles, so the
                    # PE sees only one tile_size transition per run (tile
                    # switches stall the LDWEIGHTS pipeline)
                    i = 0
                    for kt, ku in blocks:
                        tp = t + kt - 1
                        bq = (d + ku - 1) * XDP
                        for kv in range(3):
                            nc.tensor.matmul(
                                ps[:, :, :], w1p[:, _g27(kt, ku, kv), :],
                                xa[:, tp, bq + kv * XROW:bq + kv * XROW + N1],
                                start=(i == 0), stop=False)
                            i += 1
                        nc.tensor.matmul(
                            ps[:, :, :], w1pb[:, kt * 3 + ku, :],
                            xb[:, tp, bq:bq + N1],
                            start=False, stop=False)
                        i += 1
                    for i, (kt, ku) in enumerate(blocks):
                        tp = t + kt - 1
                        bq = (d + ku - 1) * XDP
                        nc.tensor.matmul(
                            ps[:, :, :], w1s[0:64, kt * 3 + ku, :],
                            xb[0:64, tp, bq + 68:bq + 68 + N1],
                            start=False, stop=(i == len(blocks) - 1))
                    nc.scalar.activation(
                        ht[:, t, d + 1, :, 1:33], ps[:, :, 1:33],
                        mybir.ActivationFunctionType.Relu, bias=b1[:, 0:1])
                # zero out-of-image h halo rows (mask is 0 only on edge cores)
                nc.vector.tensor_scalar_mul(
                    ht[:, t, :, 0, 1:33], ht[:, t, :, 0, 1:33], mt[:, 0:1])
                nc.vector.tensor_scalar_mul(
                    ht[:, t, :, HHH - 1, 1:33], ht[:, t, :, HHH - 1, 1:33],
                    mb[:, 0:1])

            # ---- conv2 ----
            # runs: edge d=0 and d=15 alone (N=256, zero-pad taps skipped),
            # interior d as 7 pairs (N=512). Taps alternate between PE column
            # groups (psum partitions 0:64 / 64:128) so adjacent matmuls run
            # concurrently; halves summed via Scalar+DVE into the stage tile.
            runs = [(0, 1)] + [(d0, 2) for d0 in range(1, 15, 2)] + [(15, 1)]
            for t in range(T):
                for d0, nd in runs:
                    taps = [(kt, ku, kv, kw) for kt in _t_taps(t)
                            for ku in range(3) if 0 < d0 + ku < 17 or nd == 2
                            for kv in range(3) for kw in range(3)]
                    nn = nd * SH * W
                    lo = taps[0::2]
                    hi = taps[1::2]
                    ps = p2.tile([128, N2], fp32)
                    for i in range(len(lo)):
                        for half, base, tp_pos in ((lo, 0, (0, 0)),
                                                   (hi, 64, (0, 64))):
                            if i >= len(half):
                                continue
                            kt, ku, kv, kw = half[i]
                            gi = _g81(kt, ku, kv, kw)
                            rhs = ht[:, t + kt - 1, d0 + ku:d0 + ku + nd,
                                     kv:kv + SH, kw:kw + W]
                            nc.tensor.matmul(
                                ps[base:base + 64, 0:nn], w2[:, gi, :], rhs,
                                start=(i == 0), stop=(i == len(half) - 1),
                                tile_position=tp_pos)
                    st = stp.tile([64, N2], fp32)
                    nc.scalar.activation(
                        st[:, 0:nn], ps[64:128, 0:nn],
                        mybir.ActivationFunctionType.Identity, bias=b2[:, 0:1])
                    nc.vector.tensor_add(st[:, 0:nn], st[:, 0:nn],
                                         ps[0:64, 0:nn])
                    nc.sync.dma_start(
                        y_d.ap()[:, t, d0 * SH * W:d0 * SH * W + nn],
                        st[:, 0:nn])
    nc.compile()
    return nc


def kernel(x, w1, b1, w2, b2):
    from concourse.bass_utils import run_bass_kernel_spmd

    hostd = _make_host_arrays(x, w1, b1, w2, b2)
    if "nc" not in _cache:
        _cache["nc"] = _build_module()
    nc = _cache["nc"]

    in_maps = []
    for core in range(NCORES):
        in_maps.append({
            "x": hostd["X"][core], "mt": hostd["MT"][core],
            "mb": hostd["MB"][core],
            "w1p": hostd["W1P"], "w1pb": hostd["W1PB"],
            "w1s": hostd["W1S"], "w2": hostd["W2"],
            "b1": hostd["B1"], "b2": hostd["B2"],
        })
    res = run_bass_kernel_spmd(nc, in_maps, core_ids=list(range(NCORES)))

    y = np.zeros((B, C_OUT, T, D, H, W), np.float32)
    for core in range(NCORES):
        b, j = divmod(core, NJ)
        yc = res.results[core]["y"].reshape(C_OUT, T, D, SH, W)
        y[b, :, :, :, SH * j:SH * (j + 1), :] = yc
    return y
